# revision 12
# baseline (speedup 1.0000x reference)
"""Trainium2 Bass kernel for the soft-clusterator problem.

reference semantics (N=200000, D=256, K=64, num_iter=11):
    data_norm = embeds / max(||embeds||_row, eps)
    mu = init
    repeat num_iter:
        mu_norm = mu / max(||mu||_row, eps)
        r = softmax(beta * data_norm @ mu_norm.T, axis=1)   # [N, K]
        mu = (r.T @ embeds) / r.sum(0)[:, None]
    return mu, r

Distribution: rows (N) sharded over 8 cores; [K, D+1] cluster stats
all-reduced on-device each iteration.  All matmul operands fp16, all
accumulation f32.  Final division mu = cm/cr done on host (with exact
compensation for the zero-padded rows, which contribute exactly 1/64
to each cluster count and nothing to cluster means).
"""

import sys
import numpy as np

for _p in ("/opt/trn_rl_repo",):
    if _p not in sys.path:
        sys.path.insert(0, _p)

from concourse import bass, bacc, tile  # noqa: E402
from concourse import bass_utils  # noqa: E402

mybir = bass.mybir

F32 = mybir.dt.float32
F16 = mybir.dt.float16
AF = mybir.ActivationFunctionType
ALU = mybir.AluOpType
AX = mybir.AxisListType

D = 256
K = 64
GROUP = 4  # row-tiles per processing group (dist tile = [128, GROUP*K])


def build(n_cores: int, tiles_per_core: int, num_iter: int):
    """Build + compile the Bass program. Returns the Bacc object."""
    T = tiles_per_core
    assert T % GROUP == 0
    NG = T // GROUP
    ROWS = T * 128

    nc = bacc.Bacc(
        "TRN2",
        target_bir_lowering=False,
        debug=False,
        num_devices=n_cores,
    )

    emb = nc.dram_tensor("embeds", [ROWS, D], F32, kind="ExternalInput")
    init_t = nc.dram_tensor("init", [K, D], F32, kind="ExternalInput")
    beta_t = nc.dram_tensor("beta", [K, 1], F32, kind="ExternalInput")
    ident_t = nc.dram_tensor("ident", [128, 128], F16, kind="ExternalInput")
    r_out = nc.dram_tensor("r_out", [ROWS, K], F32, kind="ExternalOutput")
    stats_out = nc.dram_tensor("stats_out", [K, D + 1], F32, kind="ExternalOutput")

    rg = [list(range(n_cores))]

    with tile.TileContext(nc) as tc:
        with tc.tile_pool(name="const", bufs=1) as constp, \
             tc.tile_pool(name="res", bufs=1) as resp, \
             tc.tile_pool(name="mu", bufs=2) as mup, \
             tc.tile_pool(name="mut", bufs=2) as mutp, \
             tc.tile_pool(name="stream", bufs=3) as streamp, \
             tc.tile_pool(name="soft", bufs=3) as softp, \
             tc.tile_pool(name="pdist", bufs=3, space="PSUM") as pdist, \
             tc.tile_pool(name="pstats", bufs=2, space="PSUM") as pstats, \
             tc.tile_pool(name="paux", bufs=2, space="PSUM") as paux, \
             tc.tile_pool(name="dram1", bufs=1, space="DRAM") as dramp, \
             tc.tile_pool(name="dram2", bufs=2, space="DRAM") as dramp2:

            # ---------------- constants ----------------
            idt = constp.tile([128, 128], F16, tag="idt")
            nc.sync.dma_start(idt[:], ident_t[:])
            onesc = constp.tile([128, 1], F16, tag="onesc")
            nc.vector.memset(onesc[:], 1.0)
            betasb = constp.tile([K, 1], F32, tag="betasb")
            nc.sync.dma_start(betasb[:], beta_t[:])
            lnb = constp.tile([K, 1], F32, tag="lnb")
            nc.scalar.activation(lnb[:], betasb[:], AF.Ln)
            eps6 = constp.tile([128, 1], F32, tag="eps6")
            nc.vector.memset(eps6[:], 1e-6)
            eps12 = constp.tile([K, 1], F32, tag="eps12")
            nc.vector.memset(eps12[:], 1e-12)

            # resident transposed+normalized data: [128(d), chunk, row]
            dataT = resp.tile([128, 2, ROWS], F16, tag="dataT")
            # natural fp16 copy of raw embeds, streamed back per iteration
            nat_dram = dramp.tile([NG, 128, GROUP * D], F16, tag="nat")

            # ---------------- init: cast, norms, transpose ----------------
            with tc.tile_pool(name="ini", bufs=3) as initp, \
                 tc.tile_pool(name="ini2", bufs=2) as initp2:
                for g in range(NG):
                    natg = initp2.tile([128, GROUP, D], F16, tag="natg")
                    ssg = initp.tile([128, GROUP], F32, tag="ssg")
                    for j in range(GROUP):
                        t = g * GROUP + j
                        x32 = initp.tile([128, D], F32, tag="x32")
                        nc.sync.dma_start(x32[:], emb[t * 128:(t + 1) * 128, :])
                        nc.vector.tensor_copy(natg[:, j, :], x32[:])
                    nc.sync.dma_start(
                        nat_dram[g], natg[:].rearrange("p a b -> p (a b)")
                    )
                    # row sums of squares (fp16 squares, f32 accumulate)
                    sqs = initp.tile([128, D], F16, tag="sqs")
                    for j in range(GROUP):
                        nc.scalar.activation(
                            sqs[:], natg[:, j, :], AF.Square,
                            accum_out=ssg[:, j:j + 1],
                        )
                    # inv = (ss + 1e-6) ** -0.5  via exp(-0.5 * ln(ss + eps))
                    lssg = initp.tile([128, GROUP], F32, tag="lssg")
                    nc.scalar.activation(lssg[:], ssg[:], AF.Ln, bias=eps6[:, 0:1])
                    invg = initp.tile([128, GROUP], F32, tag="invg")
                    nc.scalar.activation(invg[:], lssg[:], AF.Exp, scale=-0.5)
                    # normalized rows
                    xnn = initp2.tile([128, GROUP, D], F16, tag="xnn")
                    for j in range(GROUP):
                        nc.vector.tensor_scalar_mul(
                            xnn[:, j, :], natg[:, j, :], invg[:, j:j + 1]
                        )
                    # transpose via PE (regular matmul against identity)
                    for c in range(2):
                        trp = paux.tile([128, GROUP * 128], F32, tag="trp")
                        for j in range(GROUP):
                            nc.tensor.matmul(
                                trp[:, j * 128:(j + 1) * 128],
                                lhsT=xnn[:, j, c * 128:(c + 1) * 128],
                                rhs=idt[:],
                                start=True, stop=True,
                            )
                        dst = dataT[:, c, g * GROUP * 128:(g + 1) * GROUP * 128]
                        if c == 0:
                            nc.vector.tensor_copy(dst, trp[:])
                        else:
                            nc.scalar.copy(dst, trp[:])

            # ---------------- iterations ----------------
            cm = mup.tile([K, D], F32, tag="cm")
            nc.sync.dma_start(cm[:], init_t[:])

            for it in range(num_iter):
                last = it == num_iter - 1
                # ---- mu_norm (scaled by beta), transposed ----
                musq = mutp.tile([K, D], F32, tag="musq")
                nc.vector.tensor_mul(musq[:], cm[:], cm[:])
                ssq = mutp.tile([K, 1], F32, tag="ssq")
                nc.vector.tensor_reduce(ssq[:], musq[:], axis=AX.X, op=ALU.add)
                lssq = mutp.tile([K, 1], F32, tag="lssq")
                nc.scalar.activation(lssq[:], ssq[:], AF.Ln, bias=eps12[:, 0:1])
                rs = mutp.tile([K, 1], F32, tag="rs")
                # exp(-0.5*ln(ssq) + ln(beta)) = beta / sqrt(ssq)
                nc.scalar.activation(
                    rs[:], lssq[:], AF.Exp, scale=-0.5, bias=lnb[:, 0:1]
                )
                mun = mutp.tile([K, D], F16, tag="mun")
                nc.vector.tensor_scalar_mul(mun[:], cm[:], rs[:, 0:1])
                munT = mutp.tile([128, 2, K], F16, tag="munT")
                for c in range(2):
                    mtp = paux.tile([128, GROUP * 128], F32, tag="trp")
                    nc.tensor.matmul(
                        mtp[:, 0:K],
                        lhsT=mun[:, c * 128:(c + 1) * 128],
                        rhs=idt[0:K, 0:K],
                        start=True, stop=True,
                    )
                    nc.scalar.copy(munT[:, c, :], mtp[:, 0:K])

                # ---- main pass over row groups ----
                stp = pstats.tile([K, D + 1], F32, tag="stp")
                for g in range(NG):
                    natg = streamp.tile([128, GROUP, D + 1], F16, tag="stream")
                    nc.sync.dma_start(
                        natg[:, :, 0:D],
                        nat_dram[g].rearrange("p (a b) -> p a b", a=GROUP),
                    )
                    nc.vector.memset(natg[:, :, D:D + 1], 1.0)
                    dp = pdist.tile([128, GROUP * K], F32, tag="dp")
                    for j in range(GROUP):
                        t = g * GROUP + j
                        for c in range(2):
                            nc.tensor.matmul(
                                dp[:, j * K:(j + 1) * K],
                                lhsT=dataT[:, c, t * 128:(t + 1) * 128],
                                rhs=munT[:, c, :],
                                start=(c == 0), stop=(c == 1),
                            )
                    e = softp.tile([128, GROUP, K], F32, tag="e")
                    nc.scalar.activation(
                        e[:].rearrange("p a b -> p (a b)"), dp[:], AF.Exp
                    )
                    sums = softp.tile([128, GROUP], F32, tag="sums")
                    nc.vector.tensor_reduce(sums[:], e[:], axis=AX.X, op=ALU.add)
                    rcp = softp.tile([128, GROUP], F32, tag="rcp")
                    nc.vector.reciprocal(rcp[:], sums[:])
                    rb = softp.tile([128, GROUP, K], F16, tag="rb")
                    for j in range(GROUP):
                        nc.vector.tensor_scalar_mul(
                            rb[:, j, :], e[:, j, :], rcp[:, j:j + 1]
                        )
                    for j in range(GROUP):
                        t = g * GROUP + j
                        nc.tensor.matmul(
                            stp[:],
                            lhsT=rb[:, j, :],
                            rhs=natg[:, j, :],
                            start=(t == 0), stop=(t == T - 1),
                            skip_group_check=True,
                        )
                    if last:
                        rf = softp.tile([128, GROUP, K], F32, tag="rf")
                        for j in range(GROUP):
                            nc.vector.tensor_scalar_mul(
                                rf[:, j, :], e[:, j, :], rcp[:, j:j + 1]
                            )
                        nc.sync.dma_start(
                            r_out[g * GROUP * 128:(g + 1) * GROUP * 128, :]
                            .rearrange("(a p) b -> p a b", p=128),
                            rf[:],
                        )

                # ---- reduce stats across cores ----
                sts = mup.tile([K, D + 1], F32, tag="sts")
                nc.vector.tensor_copy(sts[:], stp[:])
                ar_in = dramp2.tile([K, D + 1], F32, tag="arin")
                ar_out = dramp2.tile([K, D + 1], F32, tag="arout")
                nc.sync.dma_start(ar_in[:], sts[:])
                nc.gpsimd.collective_compute(
                    "AllReduce",
                    ALU.add,
                    replica_groups=rg,
                    ins=[ar_in.opt()],
                    outs=[ar_out.opt()],
                )
                if last:
                    nc.sync.dma_start(stats_out[:], ar_out[:])
                else:
                    cm = mup.tile([K, D], F32, tag="cm")
                    nc.sync.dma_start(cm[:], ar_out[:, 0:D])

    nc.compile()
    return nc


_CACHE: dict = {}


def _get(n_cores, tiles_per_core, num_iter):
    key = (n_cores, tiles_per_core, num_iter)
    if key not in _CACHE:
        _CACHE[key] = build(*key)
    return _CACHE[key]


def run(embeds, init, beta, num_iter, n_cores=8, results_out=None):
    """Run on hardware. embeds [N, D] f32 (any N), returns (mu, r)."""
    N = embeds.shape[0]
    rows_per_core = -(-N // (n_cores * GROUP * 128)) * GROUP * 128
    T = rows_per_core // 128
    NPAD = rows_per_core * n_cores
    nc = _get(n_cores, T, int(num_iter))

    xpad = np.zeros((NPAD, D), np.float32)
    xpad[:N] = embeds
    ident = np.eye(128, dtype=np.float16)
    beta64 = np.full((K, 1), float(np.asarray(beta).reshape(-1)[0]), np.float32)
    ini = np.ascontiguousarray(np.asarray(init, np.float32))

    in_maps = [
        {
            "embeds": np.ascontiguousarray(xpad[c * rows_per_core:(c + 1) * rows_per_core]),
            "init": ini,
            "beta": beta64,
            "ident": ident,
        }
        for c in range(n_cores)
    ]
    res = bass_utils.run_bass_kernel_spmd(nc, in_maps, core_ids=list(range(n_cores)))
    if results_out is not None:
        results_out.append(res)
    r_full = np.concatenate(
        [res.results[c]["r_out"] for c in range(n_cores)], axis=0
    )[:N]
    stats = res.results[0]["stats_out"].astype(np.float64)
    pad_rows = NPAD - N
    cr = stats[:, D] - pad_rows / K
    mu = (stats[:, :D] / cr[:, None]).astype(np.float32)
    return mu, np.asarray(r_full, np.float32)


def kernel(**inputs):
    embeds = np.asarray(inputs["embeds"], np.float32)
    init = np.asarray(inputs["init"], np.float32)
    beta = np.asarray(inputs["beta"], np.float32)
    num_iter = int(np.asarray(inputs["num_iter"]))
    return run(embeds, init, beta, num_iter)
